# revision 1
# baseline (speedup 1.0000x reference)
"""Trainium2 8-core kernel for nn_Attention_76347338653911 (v2).

External-attention ViT block with training-mode sync-BatchNorm:
  qv = x @ W_qv ; q,v per head
  attn = softmax((BN(q@k_extT)+bias)*scale) ; out = (attn @ BN(v)) @ W_proj + b_proj

Math restructure (same as v1):
  - BN on scores: mean/beta shift cancels in softmax ->
      softmax(alpha_a[h]*s*scores + s*bias_p),  alpha_a = gamma*rsqrt(var_a)
  - BN on v folds into the projection:
      out = U @ (alpha_v (.) W_proj) + (c_v @ W_proj + b_proj)
  - only cross-core comm: 48-float AllReduce of per-head (sum, sumsq) stats
  - score stats without materializing scores:
      sum_attn[h] = ksum . colsum(q_h);  sumsq_attn[h] = ||q_h @ L||^2

Scheduling restructure (v2):
  - v computed as v^T (x-stationary, full 128-wide PE) like q; per-channel
    v stats come free from the PSUM->SBUF copies (ACT accum) + one Square
    pass, replacing the expensive M=1 ones-matmul partition reductions.
  - v^T -> v natural via PE transposes (bf16, 1cyc/row) in the AllReduce
    window; sumsq_q via a single block-diag L2=[L 0;0 L] matmul per
    head-pair (half the matmuls of per-head L).
  - attention output computed in natural [token, head*d] layout with a
    ones-column denominator, so the softmax normalization is a per-token
    (per-partition) reciprocal+multiply -- no partition broadcasts at all.
  - U_nat -> U_T via PE transposes feeding the projection, which is
    interleaved per-token-chunk with the attention tail.

Sharding: data-parallel over batch B=64 -> 8 per core.
"""

import sys
import numpy as np

sys.path.insert(0, "/opt/trn_rl_repo")

import ml_dtypes

BF = ml_dtypes.bfloat16

# problem dims (hardcoded)
B, N, C, H, HD = 64, 196, 768, 12, 64
BL = B // 8            # batch per core
TOK = BL * N           # 1568 tokens per core
PC = 98                # p-chunk (196 = 2*98)
TCH = 392              # token free-chunk (1568 = 4*392)
SCALE = HD ** -0.5     # 0.125
BN_EPS = 1e-5
NA = float(B * N * N)        # attn BN count per head (global)
NV = float(B * N * HD)       # v BN count per head (global)

_NC_CACHE = {}
DEBUG_DUMP = False


def _build_nc(single_core_timing=False):
    import concourse.bass as bass
    import concourse.mybir as mybir
    import concourse.tile as tile
    from concourse import bacc
    from concourse.bass import broadcast_tensor_aps
    from concourse.tile import add_dep_helper

    f32 = mybir.dt.float32
    bf16 = mybir.dt.bfloat16
    AF = mybir.ActivationFunctionType
    OP = mybir.AluOpType

    ndev = 1 if single_core_timing else 8
    nc = bacc.Bacc("TRN2", target_bir_lowering=False, debug=False, num_devices=ndev)

    # ---- DRAM parameters (per-core shard views) ----
    xT_d = nc.dram_tensor("xT", [C, TOK], bf16, kind="ExternalInput")
    wq_d = nc.dram_tensor("wq", [C, C], bf16, kind="ExternalInput")
    wv_d = nc.dram_tensor("wv", [C, C], bf16, kind="ExternalInput")
    wp_d = nc.dram_tensor("wp", [C, C], bf16, kind="ExternalInput")
    bpk_d = nc.dram_tensor("bpk", [128, N + 256], bf16, kind="ExternalInput")
    fpk_d = nc.dram_tensor("fpk", [128, 4], f32, kind="ExternalInput")
    rpk_d = nc.dram_tensor("rpk", [1, 24 * 3 + H + C], f32, kind="ExternalInput")
    R_d = nc.dram_tensor("R", [H, C], bf16, kind="ExternalInput")
    out_d = nc.dram_tensor("out", [TOK, C], f32, kind="ExternalOutput")
    if DEBUG_DUMP:
        dSg_d = nc.dram_tensor("dSg", [1, 48], f32, kind="ExternalOutput")
        dprm_d = nc.dram_tensor("dprm", [1, 156], f32, kind="ExternalOutput")
        dexpt_d = nc.dram_tensor("dexpt", [PC, 2, N], bf16, kind="ExternalOutput")
        dvpr_d = nc.dram_tensor("dvpr", [PC, 2, H, HD + 1], bf16, kind="ExternalOutput")
        dUT_d = nc.dram_tensor("dUT", [128, 6, N], bf16, kind="ExternalOutput")
        dqT_d = nc.dram_tensor("dqT", [128, 6, N], bf16, kind="ExternalOutput")
        dvT_d = nc.dram_tensor("dvT", [128, 6, N], bf16, kind="ExternalOutput")

    with tile.TileContext(nc) as tc:
        with (
            tc.tile_pool(name="persist", bufs=1) as pp,
            tc.tile_pool(name="dram", bufs=1, space="DRAM") as dramp,
        ):
            # ---- persistent SBUF tensors ----
            wp = pp.tile([128, 6, C], bf16, tag="wp")
            weff = pp.tile([128, 6, C], bf16, tag="weff")
            bpk = pp.tile([128, N + 256], bf16, tag="bpk")
            kT = bpk[:, 0:N]           # k^T duplicated in both halves
            L2s = bpk[:, N:N + 128]    # blockdiag(L, L)
            I128 = bpk[:, N + 128:N + 256]
            fpk = pp.tile([128, 4], f32, tag="fpk")
            slhsA = fpk[:, 0:2]
            sbias = fpk[0:PC, 2:4]
            rpk = pp.tile([1, 24 * 3 + H + C], f32, tag="rpk")
            ndiv = rpk[:, 0:24]
            gam2 = rpk[:, 24:48]
            nseed = rpk[:, 48:72]
            bet = rpk[:, 72:72 + H]
            bproj = rpk[:, 72 + H:72 + H + C]
            Rs = pp.tile([H, C], bf16, tag="Rs")
            qT = pp.tile([128, 6, TOK], bf16, tag="qT")
            vpr = pp.tile([PC, 16, H, HD + 1], bf16, tag="vpr")
            U_T = pp.tile([128, 6, TOK], bf16, tag="U_T")
            qcol = pp.tile([128, 6], f32, tag="qcol")
            ysq2 = pp.tile([128, 6], f32, tag="ysq2")
            vcol = pp.tile([128, 6], f32, tag="vcol")
            vsq2 = pp.tile([128, 6], f32, tag="vsq2")
            e0 = pp.tile([128, 1], f32, tag="e0")
            e1 = pp.tile([128, 1], f32, tag="e1")
            AR = pp.tile([1, 48], f32, tag="AR")
            Sg = pp.tile([1, 48], f32, tag="Sg")
            expscb = pp.tile([PC, H], f32, tag="expscb")
            avb = pp.tile([128, H], f32, tag="avb")
            avc = pp.tile([128, 6], f32, tag="avc")
            beffb = pp.tile([128, C], f32, tag="beffb")
            cvbf = pp.tile([1, H], bf16, tag="cvbf")
            beffbf = pp.tile([1, C], bf16, tag="beffbf")
            ones1 = pp.tile([1, 128], bf16, tag="ones1")
            onec = pp.tile([PC, 1], bf16, tag="onec")
            cvT = pp.tile([H, 1], bf16, tag="cvT")
            prm = pp.tile([1, 13 * 12], f32, tag="prm")
            beffr = pp.tile([1, C], f32, tag="beffr")

            arin = dramp.tile([1, 48], f32)
            arout = dramp.tile([1, 48], f32)

            vt_cm = tc.tile_pool(name="vtp", bufs=1)
            vtpool = vt_cm.__enter__()
            vT = vtpool.tile([128, 6, TOK], bf16, tag="vT")
            early_cm = tc.tile_pool(name="early", bufs=1)
            earlyp = early_cm.__enter__()
            xT = earlyp.tile([128, 6, TOK], bf16, tag="xT")
            wq = earlyp.tile([128, 6, C], bf16, tag="wq")
            wv = earlyp.tile([128, 6, C], bf16, tag="wv")

            # ---- loads: wq first (Q needs all of it), then xT per kc ----
            for kc in range(3):
                nc.sync.dma_start(
                    wq[:, kc:kc + 1, :],
                    wq_d.ap().rearrange("(o p) t -> p o t", p=128)[:, kc:kc + 1, :])
                nc.sync.dma_start(
                    xT[:, kc, :],
                    xT_d.ap().rearrange("(o p) t -> p o t", p=128)[:, kc, :])
            nc.sync.dma_start(
                wq[:, 3:6, :],
                wq_d.ap().rearrange("(o p) t -> p o t", p=128)[:, 3:6, :])
            for kc in range(3, 6):
                nc.sync.dma_start(
                    xT[:, kc, :],
                    xT_d.ap().rearrange("(o p) t -> p o t", p=128)[:, kc, :])
            nc.sync.dma_start(bpk[:], bpk_d.ap())
            nc.sync.dma_start(fpk[:], fpk_d.ap())
            nc.sync.dma_start(rpk[:], rpk_d.ap())
            for half2 in range(2):
                nc.sync.dma_start(
                    wv[:, 3 * half2:3 * half2 + 3, :],
                    wv_d.ap().rearrange("(o p) t -> p o t", p=128)
                    [:, 3 * half2:3 * half2 + 3, :])

            nc.gpsimd.memset(vpr[:, :, :, HD:HD + 1], 1.0)
            nc.gpsimd.memset(e0[:], 0.0)
            nc.gpsimd.memset(e0[0:64, :], 1.0)
            nc.gpsimd.memset(e1[:], 0.0)
            nc.gpsimd.memset(e1[64:128, :], 1.0)
            nc.gpsimd.memset(AR[:], 0.0)
            nc.gpsimd.memset(ones1[:], 1.0)
            nc.gpsimd.memset(onec[:], 1.0)

            def r4(ap):  # [p, 1568] -> [p, 4, 392]
                return ap.rearrange("p (a b) -> p a b", a=4)

            def r2(ap):  # [p, 768] -> [p, 2, 384]
                return ap.rearrange("p (a b) -> p a b", a=2)

            # dummy Exp: pull the exp table-set load off the critical path
            dumm = pp.tile([1, 16], f32, tag="dumm")
            nc.vector.memset(dumm[:], 0.0)
            nc.scalar.activation(dumm[:], dumm[:], AF.Exp)

            # ============ Phase Q: q^T = (x @ Wq)^T, + col sums ==========
            with (
                tc.tile_pool(name="psqv", bufs=2, space="PSUM") as qvpool,
                tc.tile_pool(name="sqs", bufs=2) as sq_pool,
            ):
                for ht in range(6):
                    qp = qvpool.tile([128, 4, 512], f32, tag="qv")
                    for ncc in range(4):
                        for kc in range(6):
                            nc.tensor.matmul(
                                qp[:, ncc, :TCH],
                                wq[:, kc, ht * 128:(ht + 1) * 128],
                                xT[:, kc, ncc * TCH:(ncc + 1) * TCH],
                                start=(kc == 0), stop=(kc == 5),
                            )
                    nc.scalar.activation(
                        r4(qT[:, ht, :]), qp[:, :, :TCH],
                        AF.Identity, accum_out=qcol[:, ht:ht + 1],
                    )

                # == Phases Y+V interleaved: ysq group g then v^T group g ==
                for g in range(6):
                    yp = qvpool.tile([128, 4, 512], f32, tag="qv")
                    for ncc in range(4):
                        nc.tensor.matmul(
                            yp[:, ncc, :TCH], L2s[:],
                            qT[:, g, ncc * TCH:(ncc + 1) * TCH],
                            start=True, stop=True,
                        )
                    # copy+square+reduce on the otherwise-idle DVE (ACT is
                    # the pacer of this window; DVE cannot dual-read PSUM)
                    ys = sq_pool.tile([128, 4, TCH], bf16, tag="ys")
                    nc.vector.tensor_copy(ys[:], yp[:, :, :TCH])
                    ys2 = sq_pool.tile([128, 4, TCH], bf16, tag="ys2")
                    nc.vector.tensor_tensor(ys2[:], ys[:], ys[:], OP.mult)
                    nc.vector.tensor_reduce(
                        ysq2[:, g:g + 1],
                        ys2[:].rearrange("p a b -> p (a b)"),
                        axis=mybir.AxisListType.X, op=OP.add)
                    vp = qvpool.tile([128, 4, 512], f32, tag="qv")
                    for ncc in range(4):
                        for kc in range(6):
                            nc.tensor.matmul(
                                vp[:, ncc, :TCH],
                                wv[:, kc, g * 128:(g + 1) * 128],
                                xT[:, kc, ncc * TCH:(ncc + 1) * TCH],
                                start=(kc == 0), stop=(kc == 5),
                            )
                    nc.scalar.activation(
                        r4(vT[:, g, :]), vp[:, :, :TCH],
                        AF.Identity, accum_out=vcol[:, g:g + 1],
                    )
                    vs = sq_pool.tile([128, 4, TCH], bf16, tag="ys")
                    nc.scalar.activation(
                        vs[:], r4(vT[:, g, :]), AF.Square,
                        accum_out=vsq2[:, g:g + 1],
                    )


            # ====== Phase T: v^T -> v natural via PE transposes ==========
            # (kc 0-2 run while the stats wait on the ACT tail; kc 3-5 fill
            # the AllReduce window)
            tp_cm = tc.tile_pool(name="pst", bufs=2, space="PSUM")
            tpool = tp_cm.__enter__()

            def emit_vtr(kc):
                for th in range(2):  # 8 token chunks per tile
                    vtp = tpool.tile([PC, 8, 128], bf16, tag="vtp")
                    for j in range(8):
                        t = th * 8 + j
                        nc.tensor.transpose(
                            vtp[:, j, :],
                            vT[:, kc, t * PC:(t + 1) * PC],
                            I128[:],
                        )
                    dst = vpr[:, th * 8:th * 8 + 8, 2 * kc:2 * kc + 2, 0:HD]
                    src = vtp[:].rearrange("p a (h d) -> p a h d", h=2)
                    if (kc * 2 + th) % 2 == 0:
                        nc.vector.tensor_copy(dst, src)
                    else:
                        nc.scalar.activation(dst, src, AF.Identity)

            for kc in range(3):
                emit_vtr(kc)

            # ============== Phase S: fold stats, AllReduce ===============
            with tc.tile_pool(name="pss", bufs=1, space="PSUM") as spool:
                psA = spool.tile([1, 512], f32, tag="psA")
                nc.tensor.matmul(psA[:, 0:6], slhsA[:, 0:1], qcol[:], start=True, stop=True)
                nc.tensor.matmul(psA[:, 8:14], slhsA[:, 1:2], qcol[:], start=True, stop=True)
                nc.tensor.matmul(psA[:, 16:22], e0[:], ysq2[:], start=True, stop=True)
                nc.tensor.matmul(psA[:, 24:30], e1[:], ysq2[:], start=True, stop=True)
                nc.tensor.matmul(psA[:, 32:38], e0[:], vcol[:], start=True, stop=True)
                nc.tensor.matmul(psA[:, 40:46], e1[:], vcol[:], start=True, stop=True)
                nc.tensor.matmul(psA[:, 48:54], e0[:], vsq2[:], start=True, stop=True)
                nc.tensor.matmul(psA[:, 56:62], e1[:], vsq2[:], start=True, stop=True)
                for blk in range(4):
                    nc.vector.tensor_copy(
                        AR[0:1, 12 * blk:12 * blk + 12]
                        .rearrange("p (c a) -> p c a", a=2),
                        psA[:, 16 * blk:16 * blk + 16]
                        .rearrange("p (a c) -> p c a", a=2)[:, 0:6, :],
                    )

            early_cm.__exit__(None, None, None)

            nc.sync.dma_start(arin[:], AR[:])
            if single_core_timing:
                nc.sync.dma_start(arout[:], arin[:])
            else:
                nc.gpsimd.collective_compute(
                    "AllReduce", OP.add,
                    ins=[arin.opt()], outs=[arout.opt()],
                    replica_groups=[list(range(8))],
                )
            nc.sync.dma_start(Sg[:], arout[:])

            # W_proj / R loads deferred to here: needed only after the AR
            for half2 in range(2):
                nc.sync.dma_start(
                    wp[:, 3 * half2:3 * half2 + 3, :],
                    wp_d.ap().rearrange("(o p) t -> p o t", p=128)
                    [:, 3 * half2:3 * half2 + 3, :])
            nc.sync.dma_start(Rs[:], R_d.ap())

            # ============== Phase P: BN affine params (batched a|v) ======
            def m24(i):
                return prm[:, i * 24:(i + 1) * 24]

            mean_av, ex2_av, var_av, rstd_av, alpha_av, tmp_av = (
                m24(i) for i in range(6))
            expsc = prm[:, 144:156]

            Sg4 = Sg[:].rearrange("p (a b c) -> p a b c", a=2, b=2)

            def v24(ap):
                return ap.rearrange("p (a c) -> p a c", a=2)

            nc.vector.tensor_tensor(v24(mean_av), Sg4[:, :, 0, :], v24(ndiv[:]), OP.mult)
            nc.vector.tensor_tensor(v24(ex2_av), Sg4[:, :, 1, :], v24(ndiv[:]), OP.mult)
            nc.vector.tensor_tensor(var_av, mean_av, mean_av, OP.mult)
            nc.vector.tensor_sub(var_av, ex2_av, var_av)
            nc.vector.tensor_scalar_add(var_av, var_av, BN_EPS)
            # rstd = rsqrt(var), Newton iters from constant seeds
            nc.vector.tensor_copy(rstd_av, nseed[:])
            for _ in range(2):
                nc.vector.tensor_tensor(tmp_av, rstd_av, rstd_av, OP.mult)
                nc.vector.tensor_tensor(tmp_av, var_av, tmp_av, OP.mult)
                nc.vector.tensor_scalar(tmp_av, tmp_av, -0.5, 1.5, OP.mult, OP.add)
                nc.vector.tensor_tensor(rstd_av, rstd_av, tmp_av, OP.mult)
            nc.vector.tensor_tensor(alpha_av, gam2[:], rstd_av, OP.mult)
            nc.vector.tensor_scalar_mul(expsc, alpha_av[:, 0:12], SCALE)
            nc.gpsimd.partition_broadcast(expscb[:], expsc)

            cv = tmp_av[:, 0:12]  # reuse scratch (newton done)
            nc.vector.tensor_tensor(cv, mean_av[:, 12:24], alpha_av[:, 12:24], OP.mult)
            nc.vector.tensor_sub(cv, bet[:], cv)
            nc.vector.tensor_copy(cvbf[:], cv)
            nc.gpsimd.partition_broadcast(avb[:], alpha_av[:, 12:24])
            nc.vector.tensor_copy(avc[0:64, :], avb[0:64, 0:12:2])
            nc.vector.tensor_copy(avc[64:128, :], avb[64:128, 1:12:2])
            for t in range(6):
                nc.vector.tensor_scalar_mul(
                    weff[:, t, :], wp[:, t, :], avc[:, t:t + 1]
                )

            vt_cm.__exit__(None, None, None)

            exptp_cm = tc.tile_pool(name="exptp", bufs=1)
            exptp = exptp_cm.__enter__()
            expt = exptp.tile([PC, 2, H, TOK], bf16, tag="expt")

            for kc in range(3, 6):
                emit_vtr(kc)
            tp_cm.__exit__(None, None, None)

            # ==== b_eff = c_v @ W_proj + b_proj (R = head-rowsums of Wp);
            # cv transposed [1,12]->[12,1] on the PE (after the v-transposes
            # in PE order, so no head-of-line stall in the cost model).
            with tc.tile_pool(name="psbep", bufs=1, space="PSUM") as beppool:
                cvp = beppool.tile([H, 1], bf16, tag="bep")
                nc.tensor.transpose(cvp[:], cvbf[:], I128[0:1, 0:1])
                nc.scalar.activation(cvT[:], cvp[:], AF.Identity)
                bep = beppool.tile([1, 2, 512], f32, tag="bep")
                for n2 in range(2):
                    nc.tensor.matmul(
                        bep[:, n2, :384], cvT[:], Rs[:, n2 * 384:(n2 + 1) * 384],
                        start=True, stop=True,
                    )
                nc.vector.tensor_tensor(
                    r2(beffr[:]), bep[:, :, :384], r2(bproj[:]), OP.add)
                nc.vector.tensor_copy(beffbf[:], beffr[:])
                nc.gpsimd.partition_broadcast(beffb[:], beffr[:])

            # ======== Phase A + O, interleaved by token half ============
            # psum: sc 2x2 + opx(shared op/utp) 1x2 + pmm 1x2 = 8 banks
            HB = BL // 2
            rec_insts = []   # per-strip custom recip instr (for WAR edges)
            mult_insts = []  # per-strip normalize mults (2 each)
            emitted_m = [0]  # next O m-tile to emit

            with (
                tc.tile_pool(name="psop", bufs=4, space="PSUM") as oppool,
                tc.tile_pool(name="pssc", bufs=4, space="PSUM") as scpool,
                tc.tile_pool(name="unp", bufs=3) as unpool,
                tc.tile_pool(name="dnp", bufs=4) as dnpool,
                tc.tile_pool(name="rcp", bufs=4) as rcpool,
                tc.tile_pool(name="ostp", bufs=3) as ostpool,
            ):
                DNB = 4  # dn/rec pool bufs

                o_ready = [0]

                def emit_o_tiles(upto_tok):
                    o_ready[0] = max(o_ready[0], upto_tok)
                    while emitted_m[0] * 128 + 128 <= upto_tok or (
                            emitted_m[0] == 12 and upto_tok >= TOK):
                        emit_o_tile()

                def emit_o_one():
                    if (emitted_m[0] * 128 + 128 <= o_ready[0] or (
                            emitted_m[0] == 12 and o_ready[0] >= TOK)):
                        emit_o_tile()

                def emit_o_tile():
                        m = emitted_m[0]
                        emitted_m[0] += 1
                        rows = 128 if m < 12 else 32
                        tail = m >= 9
                        ost = ostpool.tile([128, C], f32, tag="ost")
                        for n2 in range(2):
                            pmm = oppool.tile([128, 1, 512], f32, tag="opx")
                            for kc in range(6):
                                nc.tensor.matmul(
                                    pmm[:rows, 0, :384],
                                    U_T[:, kc, m * 128:m * 128 + rows],
                                    weff[:, kc, n2 * 384:(n2 + 1) * 384],
                                    start=(kc == 0),
                                    stop=(kc == 5 and not tail),
                                )
                            if tail:
                                # fold b_eff in as a K=1 rank-1 update so the
                                # psum->sbuf copy can run on the idle ACT
                                nc.tensor.matmul(
                                    pmm[:rows, 0, :384],
                                    ones1[0:1, 0:rows],
                                    beffbf[0:1, n2 * 384:(n2 + 1) * 384],
                                    start=False, stop=True,
                                )
                                nc.scalar.activation(
                                    ost[:rows, n2 * 384:(n2 + 1) * 384],
                                    pmm[:rows, 0, :384], AF.Identity)
                            else:
                                nc.vector.tensor_tensor(
                                    ost[:rows, n2 * 384:(n2 + 1) * 384],
                                    pmm[:rows, 0, :384],
                                    beffb[:rows, n2 * 384:(n2 + 1) * 384],
                                    OP.add,
                                )
                        if m >= 11:
                            for n2 in range(2):
                                nc.sync.dma_start(
                                    out_d.ap()[m * 128:m * 128 + rows,
                                               n2 * 384:(n2 + 1) * 384],
                                    ost[:rows, n2 * 384:(n2 + 1) * 384])
                        else:
                            nc.sync.dma_start(
                                out_d.ap()[m * 128:m * 128 + rows, :],
                                ost[:rows, :])

                SEGS = ((0, 0, 392), (1, 392, 392), (2, 784, 392),
                        (3, 1176, 392), (4, 1568, 0))

                def emit_sc(seg, tok0, ntok, h):
                    nt = min(TCH, ntok)
                    qb = (h % 2) * 64
                    for pc in range(2):
                        sp = scpool.tile([PC, 1, 512], f32, tag="sc")
                        nc.tensor.matmul(
                            sp[:, 0, :nt],
                            kT[qb:qb + 64, pc * PC:(pc + 1) * PC],
                            qT[qb:qb + 64, h // 2, tok0:tok0 + nt],
                            start=True, stop=True,
                        )
                        nc.scalar.activation(
                            expt[:, pc, h, tok0:tok0 + nt],
                            sp[:, 0, :nt], AF.Exp,
                            bias=sbias[:, pc:pc + 1],
                            scale=expscb[0:PC, h:h + 1],
                        )

                for seg, tok0, ntok in SEGS:
                    # previous segment's batches: their attn@v interleaves
                    # with this segment's score/exp stream, a few heads per av
                    pseg = SEGS[seg - 1] if seg > 0 else None
                    pbs = (list(range(pseg[1] // N, (pseg[1] + pseg[2]) // N))
                           if pseg else [])
                    nh = H if ntok else 0
                    stride = max(1, nh // max(1, len(pbs)))
                    sched = []
                    bi = 0
                    for h in range(nh):
                        sched.append(("sc", h))
                        if (h + 1) % stride == 0 and bi < len(pbs):
                            sched.append(("av", pbs[bi]))
                            bi += 1
                    while bi < len(pbs):
                        sched.append(("av", pbs[bi]))
                        bi += 1

                    for kind, idx in sched:
                        if kind == "sc":
                            emit_sc(seg, tok0, ntok, idx)
                            emit_o_one()
                            continue
                        b = idx
                        un = unpool.tile([PC, 2, C], bf16, tag="un")
                        for strip in range(2):
                            k = (b * 2 + strip)
                            rec = rcpool.tile([PC, H], f32, tag="rec")
                            ops_ = []
                            # denominators as N=1 matmuls into one psum tile:
                            # slots stay contiguous and no psum->sbuf copy
                            den = oppool.tile([PC, 16], f32, tag="opx")
                            dmm = None
                            for bank in range(2):
                                nhd = 7 if bank == 0 else 5
                                op = oppool.tile([PC, 1, 512], f32, tag="opx")
                                for hl in range(nhd):
                                    h = bank * 7 + hl
                                    for pc in range(2):
                                        nc.tensor.matmul(
                                            op[:, 0, 64 * hl:64 * hl + 64],
                                            expt[:, pc, h,
                                                 b * N + strip * PC:
                                                 b * N + strip * PC + PC],
                                            vpr[:, 2 * b + pc, h, 0:HD],
                                            start=(pc == 0), stop=(pc == 1),
                                        )
                                        dmm = nc.tensor.matmul(
                                            den[:, h:h + 1],
                                            expt[:, pc, h,
                                                 b * N + strip * PC:
                                                 b * N + strip * PC + PC],
                                            onec[:],
                                            start=(pc == 0), stop=(pc == 1),
                                        )
                                ops_.append((op, nhd))
                            ri = nc.vector.reciprocal_approx_fast(
                                rec[:], den[:, 0:H])
                            rec_insts.append(ri)
                            # RAW: recip reads the denom psum after the mms
                            add_dep_helper(ri.ins, dmm.ins, reason="RAW recip<-denmm")
                            if k >= DNB:
                                add_dep_helper(ri.ins, mult_insts[2 * (k - DNB)].ins,
                                               reason="WAR rec reuse")
                                add_dep_helper(ri.ins, mult_insts[2 * (k - DNB) + 1].ins,
                                               reason="WAR rec reuse")
                            # normalize: U_nat = op * (1/denom), per bank
                            for bank, (op, nhd) in enumerate(ops_):
                                in1 = (op[:, 0, 0:64 * nhd]
                                       .rearrange("p (s d) -> p s d", d=HD))
                                in2 = (rec[:, bank * 7:bank * 7 + nhd]
                                       .rearrange("p (s o) -> p s o", o=1))
                                b1, b2 = broadcast_tensor_aps(in1, in2)
                                mu = nc.vector.tensor_tensor(
                                    un[:, strip, bank * 7 * HD:(bank * 7 + nhd) * HD]
                                    .rearrange("p (s d) -> p s d", d=HD),
                                    b1, b2, OP.mult,
                                )
                                add_dep_helper(mu.ins, ri.ins, reason="RAW mult<-recip")
                                mult_insts.append(mu)

                        if b >= 6:
                            # last two batches: per-batch transposes so the
                            # projection tiles unlock one batch earlier
                            for kc in range(6):
                                utp = oppool.tile([128, 2, PC], bf16, tag="opx")
                                for ss in range(2):
                                    nc.tensor.transpose(
                                        utp[:, ss, :],
                                        un[:, ss, 128 * kc:128 * kc + 128],
                                        I128[0:PC, 0:PC],
                                    )
                                dst = (U_T[:, kc, b * N:(b + 1) * N]
                                       .rearrange("p (a b) -> p a b", a=2))
                                if kc % 2 == 1:
                                    nc.scalar.activation(dst, utp[:], AF.Identity)
                                else:
                                    nc.vector.tensor_copy(dst, utp[:])
                            emit_o_tiles((b + 1) * N)
                            prev_un = un
                            continue
                        # U_nat -> U_T transposes (pair-batched); copies on
                        # DVE except the very last pair (ACT idle by then)
                        if b % 2 == 1:
                            for kc in range(6):
                                utp = oppool.tile([128, 4, PC], bf16, tag="opx")
                                for j in range(4):
                                    bb = b - 1 + j // 2
                                    ss = j % 2
                                    src = un if bb == b else prev_un
                                    nc.tensor.transpose(
                                        utp[:, j, :],
                                        src[:, ss, 128 * kc:128 * kc + 128],
                                        I128[0:PC, 0:PC],
                                    )
                                dst = (U_T[:, kc, (b - 1) * N:(b + 1) * N]
                                       .rearrange("p (a b) -> p a b", a=4))
                                if b == 7 and kc % 2 == 1:
                                    nc.scalar.activation(dst, utp[:], AF.Identity)
                                else:
                                    nc.vector.tensor_copy(dst, utp[:])
                            o_ready[0] = max(o_ready[0], (b + 1) * N)
                            emit_o_one()
                        prev_un = un

                emit_o_tiles(TOK + 1)

            exptp_cm.__exit__(None, None, None)

            if DEBUG_DUMP:
                nc.sync.dma_start(dSg_d.ap(), Sg[:])
                # (dexpt dump removed: expt pool is closed by now)
                nc.sync.dma_start(dprm_d.ap(), prm[:])
                pass
                nc.sync.dma_start(dvpr_d.ap(), vpr[:, 0:2, :, :])
                nc.sync.dma_start(dUT_d.ap(), U_T[:, :, 0:N])
                nc.sync.dma_start(dqT_d.ap(), qT[:, :, 0:N])


    nc.compile()
    return nc


def _get_nc():
    if "nc" not in _NC_CACHE:
        _NC_CACHE["nc"] = _build_nc()
    return _NC_CACHE["nc"]


def _host_prep(inputs):
    x = np.asarray(inputs["x"], np.float32)
    W_qv = np.asarray(inputs["W_qv"], np.float32)
    k_ext = np.asarray(inputs["k_ext"], np.float32)
    attn_bias = np.asarray(inputs["attn_bias"], np.float32).reshape(1, N)
    gamma = np.asarray(inputs["bn_gamma"], np.float32).reshape(1, H)
    beta = np.asarray(inputs["bn_beta"], np.float32).reshape(1, H)
    W_proj = np.asarray(inputs["W_proj"], np.float32)
    b_proj = np.asarray(inputs["b_proj"], np.float32).reshape(1, C)

    wq_bf = np.ascontiguousarray(W_qv[:, :C]).astype(BF)
    wv_bf = np.ascontiguousarray(W_qv[:, C:]).astype(BF)
    wp_bf = W_proj.astype(BF)
    kT1 = np.ascontiguousarray(k_ext.T).astype(BF)
    kT_bf = np.concatenate([kT1, kT1], axis=0)  # duplicated in both halves

    G = k_ext.astype(np.float64)
    G = G.T @ G
    L = np.linalg.cholesky(G + 1e-6 * np.eye(HD)).astype(np.float32)
    L2 = np.zeros((128, 128), np.float32)
    L2[0:64, 0:64] = L
    L2[64:128, 64:128] = L

    I128 = np.eye(128, dtype=np.float32)

    ksum = k_ext.sum(0).astype(np.float32)
    slhsA = np.zeros((128, 2), np.float32)
    slhsA[0:64, 0] = ksum
    slhsA[64:128, 1] = ksum

    sbias = np.ascontiguousarray(
        (SCALE * attn_bias.reshape(2, PC)).T
    ).astype(np.float32)

    R = W_proj.reshape(H, HD, C).sum(1).astype(np.float32)

    ndiv = np.concatenate([np.full(12, 1.0 / NA), np.full(12, 1.0 / NV)]
                          ).reshape(1, 24).astype(np.float32)
    gam2 = np.concatenate([gamma, gamma], axis=1).astype(np.float32)
    nseed = np.concatenate([np.full(12, 0.125), np.full(12, 1.0)]
                           ).reshape(1, 24).astype(np.float32)

    bpk = np.concatenate(
        [kT_bf, L2.astype(BF), I128.astype(BF)], axis=1)
    fpk = np.zeros((128, 4), np.float32)
    fpk[:, 0:2] = slhsA
    fpk[0:PC, 2:4] = sbias
    rpk = np.concatenate(
        [ndiv.reshape(-1), gam2.reshape(-1), nseed.reshape(-1),
         beta.reshape(-1), b_proj.reshape(-1)]).reshape(1, -1).astype(np.float32)
    common = dict(
        wq=wq_bf, wv=wv_bf, wp=wp_bf, bpk=bpk, fpk=fpk, rpk=rpk,
        R=R.astype(BF),
    )
    in_maps = []
    for c in range(8):
        xs = x[c * BL:(c + 1) * BL].reshape(TOK, C)
        xT = np.ascontiguousarray(xs.T).astype(BF)
        in_maps.append(dict(common, xT=xT))
    return in_maps


def kernel(**inputs):
    from concourse.bass_utils import run_bass_kernel_spmd

    in_maps = _host_prep(inputs)
    nc = _get_nc()
    res = run_bass_kernel_spmd(nc, in_maps, core_ids=list(range(8)))
    outs = [res.results[c]["out"].reshape(BL, N, C) for c in range(8)]
    return np.concatenate(outs, axis=0)



# revision 9
# speedup vs baseline: 1.1558x; 1.1558x over previous
"""Trainium2 8-core kernel for nn_Attention_76347338653911 (v2).

External-attention ViT block with training-mode sync-BatchNorm:
  qv = x @ W_qv ; q,v per head
  attn = softmax((BN(q@k_extT)+bias)*scale) ; out = (attn @ BN(v)) @ W_proj + b_proj

Math restructure (same as v1):
  - BN on scores: mean/beta shift cancels in softmax ->
      softmax(alpha_a[h]*s*scores + s*bias_p),  alpha_a = gamma*rsqrt(var_a)
  - BN on v folds into the projection:
      out = U @ (alpha_v (.) W_proj) + (c_v @ W_proj + b_proj)
  - only cross-core comm: 48-float AllReduce of per-head (sum, sumsq) stats
  - score stats without materializing scores:
      sum_attn[h] = ksum . colsum(q_h);  sumsq_attn[h] = ||q_h @ L||^2

Scheduling restructure (v2):
  - v computed as v^T (x-stationary, full 128-wide PE) like q; per-channel
    v stats come free from the PSUM->SBUF copies (ACT accum) + one Square
    pass, replacing the expensive M=1 ones-matmul partition reductions.
  - v^T -> v natural via PE transposes (bf16, 1cyc/row) in the AllReduce
    window; sumsq_q via a single block-diag L2=[L 0;0 L] matmul per
    head-pair (half the matmuls of per-head L).
  - attention output computed in natural [token, head*d] layout with a
    ones-column denominator, so the softmax normalization is a per-token
    (per-partition) reciprocal+multiply -- no partition broadcasts at all.
  - U_nat -> U_T via PE transposes feeding the projection, which is
    interleaved per-token-chunk with the attention tail.

Sharding: data-parallel over batch B=64 -> 8 per core.
"""

import sys
import numpy as np

sys.path.insert(0, "/opt/trn_rl_repo")

import ml_dtypes

BF = ml_dtypes.bfloat16

# problem dims (hardcoded)
B, N, C, H, HD = 64, 196, 768, 12, 64
BL = B // 8            # batch per core
TOK = BL * N           # 1568 tokens per core
PC = 98                # p-chunk (196 = 2*98)
TCH = 392              # token free-chunk (1568 = 4*392)
SCALE = HD ** -0.5     # 0.125
BN_EPS = 1e-5
NA = float(B * N * N)        # attn BN count per head (global)
NV = float(B * N * HD)       # v BN count per head (global)

_NC_CACHE = {}
DEBUG_DUMP = False


def _build_nc(single_core_timing=False):
    import concourse.bass as bass
    import concourse.mybir as mybir
    import concourse.tile as tile
    from concourse import bacc
    from concourse.bass import broadcast_tensor_aps
    from concourse.tile import add_dep_helper

    f32 = mybir.dt.float32
    bf16 = mybir.dt.bfloat16
    AF = mybir.ActivationFunctionType
    OP = mybir.AluOpType

    ndev = 1 if single_core_timing else 8
    nc = bacc.Bacc("TRN2", target_bir_lowering=False, debug=False, num_devices=ndev)

    fp8 = mybir.dt.float8e4
    PM = mybir.MatmulPerfMode

    # ---- DRAM parameters (per-core shard views) ----
    # fp8 DoubleRow layouts: [p, kc2, ktile, *] with contraction row
    # c = (2*kc2 + ktile)*128 + p
    x8_d = nc.dram_tensor("x8", [128, 3, 2, TOK], fp8, kind="ExternalInput")
    x8r_d = nc.dram_tensor("x8r", [128, 3, 2, TOK], fp8, kind="ExternalInput")
    wq8_d = nc.dram_tensor("wq8", [128, 3, 2, C], fp8, kind="ExternalInput")
    wv8_d = nc.dram_tensor("wv8", [128, 3, 2, C], fp8, kind="ExternalInput")
    wv8r_d = nc.dram_tensor("wv8r", [128, 3, 2, C], fp8, kind="ExternalInput")
    wp_d = nc.dram_tensor("wp", [C, C], bf16, kind="ExternalInput")
    bpk_d = nc.dram_tensor("bpk", [128, N + 256], bf16, kind="ExternalInput")
    fpk_d = nc.dram_tensor("fpk", [128, 4], f32, kind="ExternalInput")
    rpk_d = nc.dram_tensor("rpk", [1, 24 * 3 + H + C], f32, kind="ExternalInput")
    R_d = nc.dram_tensor("R", [H, C], bf16, kind="ExternalInput")
    out_d = nc.dram_tensor("out", [TOK, C], f32, kind="ExternalOutput")
    if DEBUG_DUMP:
        dSg_d = nc.dram_tensor("dSg", [1, 48], f32, kind="ExternalOutput")
        dprm_d = nc.dram_tensor("dprm", [1, 156], f32, kind="ExternalOutput")
        dexpt_d = nc.dram_tensor("dexpt", [PC, 2, N], bf16, kind="ExternalOutput")
        dvpr_d = nc.dram_tensor("dvpr", [PC, 2, H, HD + 1], bf16, kind="ExternalOutput")
        dUT_d = nc.dram_tensor("dUT", [128, 6, N], bf16, kind="ExternalOutput")
        dqT_d = nc.dram_tensor("dqT", [128, 6, N], bf16, kind="ExternalOutput")
        dvT_d = nc.dram_tensor("dvT", [128, 6, N], bf16, kind="ExternalOutput")

    with tile.TileContext(nc) as tc:
        with (
            tc.tile_pool(name="persist", bufs=1) as pp,
            tc.tile_pool(name="dram", bufs=1, space="DRAM") as dramp,
        ):
            # ---- persistent SBUF tensors ----
            wp = pp.tile([128, 6, C], bf16, tag="wp")
            weff = pp.tile([128, 6, C], bf16, tag="weff")
            bpk = pp.tile([128, N + 256], bf16, tag="bpk")
            kT = bpk[:, 0:N]           # k^T duplicated in both halves
            L2s = bpk[:, N:N + 128]    # blockdiag(L, L)
            I128 = bpk[:, N + 128:N + 256]
            fpk = pp.tile([128, 4], f32, tag="fpk")
            slhsA = fpk[:, 0:2]
            sbias = fpk[0:PC, 2:4]
            rpk = pp.tile([1, 24 * 3 + H + C], f32, tag="rpk")
            ndiv = rpk[:, 0:24]
            gam2 = rpk[:, 24:48]
            nseed = rpk[:, 48:72]
            bet = rpk[:, 72:72 + H]
            bproj = rpk[:, 72 + H:72 + H + C]
            Rs = pp.tile([H, C], bf16, tag="Rs")
            qT = pp.tile([128, 6, TOK], bf16, tag="qT")
            vpr = pp.tile([PC, 16, H, HD + 1], bf16, tag="vpr")
            U_T = pp.tile([128, 6, TOK], bf16, tag="U_T")
            qcol = pp.tile([128, 6], f32, tag="qcol")
            ysq2 = pp.tile([128, 6], f32, tag="ysq2")
            vcol = pp.tile([128, 6], f32, tag="vcol")
            vsq2 = pp.tile([128, 6], f32, tag="vsq2")
            e0 = pp.tile([128, 1], f32, tag="e0")
            e1 = pp.tile([128, 1], f32, tag="e1")
            AR = pp.tile([1, 48], f32, tag="AR")
            Sg = pp.tile([1, 48], f32, tag="Sg")
            expscb = pp.tile([PC, H], f32, tag="expscb")
            avb = pp.tile([128, H], f32, tag="avb")
            avc = pp.tile([128, 6], f32, tag="avc")
            beffb = pp.tile([128, C], f32, tag="beffb")
            cvbf = pp.tile([1, H], bf16, tag="cvbf")
            beffbf = pp.tile([1, C], bf16, tag="beffbf")
            ones1 = pp.tile([1, 128], bf16, tag="ones1")
            onec = pp.tile([PC, 1], bf16, tag="onec")
            cvT = pp.tile([H, 1], bf16, tag="cvT")
            prm = pp.tile([1, 13 * 12], f32, tag="prm")
            beffr = pp.tile([1, C], f32, tag="beffr")

            arin = dramp.tile([1, 48], f32)
            arout = dramp.tile([1, 48], f32)

            vt_cm = tc.tile_pool(name="vtp", bufs=1)
            vtpool = vt_cm.__enter__()
            vT = vtpool.tile([128, 6, TOK], bf16, tag="vT")
            early_cm = tc.tile_pool(name="early", bufs=1)
            earlyp = early_cm.__enter__()
            x8 = earlyp.tile([128, 3, 2, TOK], fp8, tag="x8")
            x8r = earlyp.tile([128, 3, 2, TOK], fp8, tag="x8r")
            wq8 = earlyp.tile([128, 3, 2, C], fp8, tag="wq8")
            wv8 = earlyp.tile([128, 3, 2, C], fp8, tag="wv8")
            wv8r = earlyp.tile([128, 3, 2, C], fp8, tag="wv8r")

            # ---- loads: wq8 + x8 first (Q), then V inputs ----
            nc.sync.dma_start(wq8[:], wq8_d.ap())
            nc.sync.dma_start(x8[:, :, :, 0:TOK // 2],
                              x8_d.ap()[:, :, :, 0:TOK // 2])
            nc.sync.dma_start(x8[:, :, :, TOK // 2:TOK],
                              x8_d.ap()[:, :, :, TOK // 2:TOK])
            nc.sync.dma_start(bpk[:], bpk_d.ap())
            nc.sync.dma_start(fpk[:], fpk_d.ap())
            nc.sync.dma_start(rpk[:], rpk_d.ap())
            nc.sync.dma_start(wv8[:], wv8_d.ap())
            nc.sync.dma_start(wv8r[:], wv8r_d.ap())
            nc.sync.dma_start(x8r[:], x8r_d.ap())

            nc.gpsimd.memset(vpr[:, :, :, HD:HD + 1], 1.0)
            nc.gpsimd.memset(e0[:], 0.0)
            nc.gpsimd.memset(e0[0:64, :], 1.0)
            nc.gpsimd.memset(e1[:], 0.0)
            nc.gpsimd.memset(e1[64:128, :], 1.0)
            nc.gpsimd.memset(AR[:], 0.0)
            nc.gpsimd.memset(ones1[:], 1.0)
            nc.gpsimd.memset(onec[:], 1.0)

            def r4(ap):  # [p, 1568] -> [p, 4, 392]
                return ap.rearrange("p (a b) -> p a b", a=4)

            def r2(ap):  # [p, 768] -> [p, 2, 384]
                return ap.rearrange("p (a b) -> p a b", a=2)

            # dummy Exp: pull the exp table-set load off the critical path
            dumm = pp.tile([1, 16], f32, tag="dumm")
            nc.vector.memset(dumm[:], 0.0)
            nc.scalar.activation(dumm[:], dumm[:], AF.Exp)

            # ============ Phase Q: q^T = (x @ Wq)^T, + col sums ==========
            with (
                tc.tile_pool(name="psqv", bufs=2, space="PSUM") as qvpool,
                tc.tile_pool(name="sqs", bufs=2) as sq_pool,
            ):
                for ht in range(6):
                    qp = qvpool.tile([128, 4, 512], f32, tag="qv")
                    for ncc in range(4):
                        for kc2 in range(3):
                            nc.tensor.matmul(
                                qp[:, ncc, :TCH],
                                wq8[:, kc2, :, ht * 128:(ht + 1) * 128],
                                x8[:, kc2, :, ncc * TCH:(ncc + 1) * TCH],
                                start=(kc2 == 0), stop=(kc2 == 2),
                                perf_mode=PM.DoubleRow,
                            )
                    nc.scalar.activation(
                        r4(qT[:, ht, :]), qp[:, :, :TCH],
                        AF.Identity, accum_out=qcol[:, ht:ht + 1],
                    )

                # == Phases Y+V interleaved: ysq group g then v^T group g ==
                for g in range(6):
                    yp = qvpool.tile([128, 4, 512], f32, tag="qv")
                    for ncc in range(4):
                        nc.tensor.matmul(
                            yp[:, ncc, :TCH], L2s[:],
                            qT[:, g, ncc * TCH:(ncc + 1) * TCH],
                            start=True, stop=True,
                        )
                    # copy+square+reduce on the otherwise-idle DVE (ACT is
                    # the pacer of this window; DVE cannot dual-read PSUM)
                    ys = sq_pool.tile([128, 4, TCH], bf16, tag="ys")
                    nc.vector.tensor_copy(ys[:], yp[:, :, :TCH])
                    ys2 = sq_pool.tile([128, 4, TCH], bf16, tag="ys2")
                    nc.vector.tensor_tensor(ys2[:], ys[:], ys[:], OP.mult)
                    nc.vector.tensor_reduce(
                        ysq2[:, g:g + 1],
                        ys2[:].rearrange("p a b -> p (a b)"),
                        axis=mybir.AxisListType.X, op=OP.add)
                    vp = qvpool.tile([128, 4, 512], f32, tag="qv")
                    for ncc in range(4):
                        ii = 0
                        for A, W in ((x8, wv8), (x8, wv8r), (x8r, wv8)):
                            for kc2 in range(3):
                                nc.tensor.matmul(
                                    vp[:, ncc, :TCH],
                                    W[:, kc2, :, g * 128:(g + 1) * 128],
                                    A[:, kc2, :, ncc * TCH:(ncc + 1) * TCH],
                                    start=(ii == 0), stop=(ii == 8),
                                    perf_mode=PM.DoubleRow,
                                )
                                ii += 1
                    nc.scalar.activation(
                        r4(vT[:, g, :]), vp[:, :, :TCH],
                        AF.Identity, scale=1.0 / 64.0,
                        accum_out=vcol[:, g:g + 1],
                    )
                    vs = sq_pool.tile([128, 4, TCH], bf16, tag="ys")
                    nc.scalar.activation(
                        vs[:], r4(vT[:, g, :]), AF.Square,
                        accum_out=vsq2[:, g:g + 1],
                    )


            # ====== Phase T: v^T -> v natural via PE transposes ==========
            # (kc 0-2 run while the stats wait on the ACT tail; kc 3-5 fill
            # the AllReduce window)
            tp_cm = tc.tile_pool(name="pst", bufs=2, space="PSUM")
            tpool = tp_cm.__enter__()

            def emit_vtr(kc):
                for th in range(2):  # 8 token chunks per tile
                    vtp = tpool.tile([PC, 8, 128], bf16, tag="vtp")
                    for j in range(8):
                        t = th * 8 + j
                        nc.tensor.transpose(
                            vtp[:, j, :],
                            vT[:, kc, t * PC:(t + 1) * PC],
                            I128[:],
                        )
                    dst = vpr[:, th * 8:th * 8 + 8, 2 * kc:2 * kc + 2, 0:HD]
                    src = vtp[:].rearrange("p a (h d) -> p a h d", h=2)
                    if (kc * 2 + th) % 2 == 0:
                        nc.vector.tensor_copy(dst, src)
                    else:
                        nc.scalar.activation(dst, src, AF.Identity)

            for kc in range(3):
                emit_vtr(kc)

            # ============== Phase S: fold stats, AllReduce ===============
            with tc.tile_pool(name="pss", bufs=1, space="PSUM") as spool:
                psA = spool.tile([1, 512], f32, tag="psA")
                nc.tensor.matmul(psA[:, 0:6], slhsA[:, 0:1], qcol[:], start=True, stop=True)
                nc.tensor.matmul(psA[:, 8:14], slhsA[:, 1:2], qcol[:], start=True, stop=True)
                nc.tensor.matmul(psA[:, 16:22], e0[:], ysq2[:], start=True, stop=True)
                nc.tensor.matmul(psA[:, 24:30], e1[:], ysq2[:], start=True, stop=True)
                nc.tensor.matmul(psA[:, 32:38], e0[:], vcol[:], start=True, stop=True)
                nc.tensor.matmul(psA[:, 40:46], e1[:], vcol[:], start=True, stop=True)
                nc.tensor.matmul(psA[:, 48:54], e0[:], vsq2[:], start=True, stop=True)
                nc.tensor.matmul(psA[:, 56:62], e1[:], vsq2[:], start=True, stop=True)
                for blk in range(4):
                    nc.vector.tensor_copy(
                        AR[0:1, 12 * blk:12 * blk + 12]
                        .rearrange("p (c a) -> p c a", a=2),
                        psA[:, 16 * blk:16 * blk + 16]
                        .rearrange("p (a c) -> p c a", a=2)[:, 0:6, :],
                    )

            early_cm.__exit__(None, None, None)

            nc.sync.dma_start(arin[:], AR[:])
            if single_core_timing:
                nc.sync.dma_start(arout[:], arin[:])
            else:
                nc.gpsimd.collective_compute(
                    "AllReduce", OP.add,
                    ins=[arin.opt()], outs=[arout.opt()],
                    replica_groups=[list(range(8))],
                )
            nc.sync.dma_start(Sg[:], arout[:])

            # W_proj / R loads deferred to here: needed only after the AR
            for half2 in range(2):
                nc.sync.dma_start(
                    wp[:, 3 * half2:3 * half2 + 3, :],
                    wp_d.ap().rearrange("(o p) t -> p o t", p=128)
                    [:, 3 * half2:3 * half2 + 3, :])
            nc.sync.dma_start(Rs[:], R_d.ap())

            # ============== Phase P: BN affine params (batched a|v) ======
            def m24(i):
                return prm[:, i * 24:(i + 1) * 24]

            mean_av, ex2_av, var_av, rstd_av, alpha_av, tmp_av = (
                m24(i) for i in range(6))
            expsc = prm[:, 144:156]

            Sg4 = Sg[:].rearrange("p (a b c) -> p a b c", a=2, b=2)

            def v24(ap):
                return ap.rearrange("p (a c) -> p a c", a=2)

            nc.vector.tensor_tensor(v24(mean_av), Sg4[:, :, 0, :], v24(ndiv[:]), OP.mult)
            nc.vector.tensor_tensor(v24(ex2_av), Sg4[:, :, 1, :], v24(ndiv[:]), OP.mult)
            nc.vector.tensor_tensor(var_av, mean_av, mean_av, OP.mult)
            nc.vector.tensor_sub(var_av, ex2_av, var_av)
            nc.vector.tensor_scalar_add(var_av, var_av, BN_EPS)
            # rstd = rsqrt(var), Newton iters from constant seeds
            nc.vector.tensor_copy(rstd_av, nseed[:])
            for _ in range(3):
                nc.vector.tensor_tensor(tmp_av, rstd_av, rstd_av, OP.mult)
                nc.vector.tensor_tensor(tmp_av, var_av, tmp_av, OP.mult)
                nc.vector.tensor_scalar(tmp_av, tmp_av, -0.5, 1.5, OP.mult, OP.add)
                nc.vector.tensor_tensor(rstd_av, rstd_av, tmp_av, OP.mult)
            nc.vector.tensor_tensor(alpha_av, gam2[:], rstd_av, OP.mult)
            nc.vector.tensor_scalar_mul(expsc, alpha_av[:, 0:12], SCALE)
            nc.gpsimd.partition_broadcast(expscb[:], expsc)

            cv = tmp_av[:, 0:12]  # reuse scratch (newton done)
            nc.vector.tensor_tensor(cv, mean_av[:, 12:24], alpha_av[:, 12:24], OP.mult)
            nc.vector.tensor_sub(cv, bet[:], cv)
            nc.vector.tensor_copy(cvbf[:], cv)
            nc.gpsimd.partition_broadcast(avb[:], alpha_av[:, 12:24])
            nc.vector.tensor_copy(avc[0:64, :], avb[0:64, 0:12:2])
            nc.vector.tensor_copy(avc[64:128, :], avb[64:128, 1:12:2])
            for t in range(6):
                nc.vector.tensor_scalar_mul(
                    weff[:, t, :], wp[:, t, :], avc[:, t:t + 1]
                )

            vt_cm.__exit__(None, None, None)

            exptp_cm = tc.tile_pool(name="exptp", bufs=1)
            exptp = exptp_cm.__enter__()
            expt = exptp.tile([PC, 2, H, TOK], bf16, tag="expt")

            for kc in range(3, 6):
                emit_vtr(kc)
            tp_cm.__exit__(None, None, None)

            # ==== b_eff = c_v @ W_proj + b_proj (R = head-rowsums of Wp);
            # cv transposed [1,12]->[12,1] on the PE (after the v-transposes
            # in PE order, so no head-of-line stall in the cost model).
            with tc.tile_pool(name="psbep", bufs=1, space="PSUM") as beppool:
                cvp = beppool.tile([H, 1], bf16, tag="bep")
                nc.tensor.transpose(cvp[:], cvbf[:], I128[0:1, 0:1])
                nc.scalar.activation(cvT[:], cvp[:], AF.Identity)
                bep = beppool.tile([1, 2, 512], f32, tag="bep")
                for n2 in range(2):
                    nc.tensor.matmul(
                        bep[:, n2, :384], cvT[:], Rs[:, n2 * 384:(n2 + 1) * 384],
                        start=True, stop=True,
                    )
                nc.vector.tensor_tensor(
                    r2(beffr[:]), bep[:, :, :384], r2(bproj[:]), OP.add)
                nc.vector.tensor_copy(beffbf[:], beffr[:])
                nc.gpsimd.partition_broadcast(beffb[:], beffr[:])

            # ======== Phase A + O, interleaved by token half ============
            # psum: sc 2x2 + opx(shared op/utp) 1x2 + pmm 1x2 = 8 banks
            HB = BL // 2
            rec_insts = []   # per-strip custom recip instr (for WAR edges)
            mult_insts = []  # per-strip normalize mults (2 each)
            emitted_m = [0]  # next O m-tile to emit

            with (
                tc.tile_pool(name="psop", bufs=4, space="PSUM") as oppool,
                tc.tile_pool(name="pssc", bufs=4, space="PSUM") as scpool,
                tc.tile_pool(name="unp", bufs=3) as unpool,
                tc.tile_pool(name="dnp", bufs=4) as dnpool,
                tc.tile_pool(name="rcp", bufs=4) as rcpool,
                tc.tile_pool(name="ostp", bufs=3) as ostpool,
            ):
                DNB = 4  # dn/rec pool bufs

                o_ready = [0]

                def emit_o_tiles(upto_tok):
                    o_ready[0] = max(o_ready[0], upto_tok)
                    while emitted_m[0] * 128 + 128 <= upto_tok or (
                            emitted_m[0] == 12 and upto_tok >= TOK):
                        emit_o_tile()

                def emit_o_one():
                    if (emitted_m[0] * 128 + 128 <= o_ready[0] or (
                            emitted_m[0] == 12 and o_ready[0] >= TOK)):
                        emit_o_tile()

                def emit_o_tile():
                        m = emitted_m[0]
                        emitted_m[0] += 1
                        rows = 128 if m < 12 else 32
                        tail = m >= 9
                        ost = ostpool.tile([128, C], f32, tag="ost")
                        for n2 in range(2):
                            pmm = oppool.tile([128, 1, 512], f32, tag="opx")
                            for kc in range(6):
                                nc.tensor.matmul(
                                    pmm[:rows, 0, :384],
                                    U_T[:, kc, m * 128:m * 128 + rows],
                                    weff[:, kc, n2 * 384:(n2 + 1) * 384],
                                    start=(kc == 0),
                                    stop=(kc == 5 and not tail),
                                )
                            if tail:
                                # fold b_eff in as a K=1 rank-1 update so the
                                # psum->sbuf copy can run on the idle ACT
                                nc.tensor.matmul(
                                    pmm[:rows, 0, :384],
                                    ones1[0:1, 0:rows],
                                    beffbf[0:1, n2 * 384:(n2 + 1) * 384],
                                    start=False, stop=True,
                                )
                                nc.scalar.activation(
                                    ost[:rows, n2 * 384:(n2 + 1) * 384],
                                    pmm[:rows, 0, :384], AF.Identity)
                            else:
                                nc.vector.tensor_tensor(
                                    ost[:rows, n2 * 384:(n2 + 1) * 384],
                                    pmm[:rows, 0, :384],
                                    beffb[:rows, n2 * 384:(n2 + 1) * 384],
                                    OP.add,
                                )
                        if m >= 11:
                            for n2 in range(2):
                                nc.sync.dma_start(
                                    out_d.ap()[m * 128:m * 128 + rows,
                                               n2 * 384:(n2 + 1) * 384],
                                    ost[:rows, n2 * 384:(n2 + 1) * 384])
                        else:
                            nc.sync.dma_start(
                                out_d.ap()[m * 128:m * 128 + rows, :],
                                ost[:rows, :])

                SEGS = ((0, 0, 392), (1, 392, 392), (2, 784, 392),
                        (3, 1176, 392), (4, 1568, 0))

                def emit_sc(seg, tok0, ntok, h):
                    nt = min(TCH, ntok)
                    qb = (h % 2) * 64
                    for pc in range(2):
                        sp = scpool.tile([PC, 1, 512], f32, tag="sc")
                        nc.tensor.matmul(
                            sp[:, 0, :nt],
                            kT[qb:qb + 64, pc * PC:(pc + 1) * PC],
                            qT[qb:qb + 64, h // 2, tok0:tok0 + nt],
                            start=True, stop=True,
                        )
                        nc.scalar.activation(
                            expt[:, pc, h, tok0:tok0 + nt],
                            sp[:, 0, :nt], AF.Exp,
                            bias=sbias[:, pc:pc + 1],
                            scale=expscb[0:PC, h:h + 1],
                        )

                for seg, tok0, ntok in SEGS:
                    # previous segment's batches: their attn@v interleaves
                    # with this segment's score/exp stream, a few heads per av
                    pseg = SEGS[seg - 1] if seg > 0 else None
                    pbs = (list(range(pseg[1] // N, (pseg[1] + pseg[2]) // N))
                           if pseg else [])
                    nh = H if ntok else 0
                    stride = max(1, nh // max(1, len(pbs)))
                    sched = []
                    bi = 0
                    for h in range(nh):
                        sched.append(("sc", h))
                        if (h + 1) % stride == 0 and bi < len(pbs):
                            sched.append(("av", pbs[bi]))
                            bi += 1
                    while bi < len(pbs):
                        sched.append(("av", pbs[bi]))
                        bi += 1

                    for kind, idx in sched:
                        if kind == "sc":
                            emit_sc(seg, tok0, ntok, idx)
                            emit_o_one()
                            continue
                        b = idx
                        un = unpool.tile([PC, 2, C], bf16, tag="un")
                        for strip in range(2):
                            k = (b * 2 + strip)
                            rec = rcpool.tile([PC, H], f32, tag="rec")
                            ops_ = []
                            # denominators as N=1 matmuls into one psum tile:
                            # slots stay contiguous and no psum->sbuf copy
                            den = oppool.tile([PC, 16], f32, tag="opx")
                            dmm = None
                            for bank in range(2):
                                nhd = 7 if bank == 0 else 5
                                op = oppool.tile([PC, 1, 512], f32, tag="opx")
                                for hl in range(nhd):
                                    h = bank * 7 + hl
                                    for pc in range(2):
                                        nc.tensor.matmul(
                                            op[:, 0, 64 * hl:64 * hl + 64],
                                            expt[:, pc, h,
                                                 b * N + strip * PC:
                                                 b * N + strip * PC + PC],
                                            vpr[:, 2 * b + pc, h, 0:HD],
                                            start=(pc == 0), stop=(pc == 1),
                                        )
                                        dmm = nc.tensor.matmul(
                                            den[:, h:h + 1],
                                            expt[:, pc, h,
                                                 b * N + strip * PC:
                                                 b * N + strip * PC + PC],
                                            onec[:],
                                            start=(pc == 0), stop=(pc == 1),
                                        )
                                ops_.append((op, nhd))
                            ri = nc.vector.reciprocal_approx_fast(
                                rec[:], den[:, 0:H])
                            rec_insts.append(ri)
                            # RAW: recip reads the denom psum after the mms
                            add_dep_helper(ri.ins, dmm.ins, reason="RAW recip<-denmm")
                            if k >= DNB:
                                add_dep_helper(ri.ins, mult_insts[2 * (k - DNB)].ins,
                                               reason="WAR rec reuse")
                                add_dep_helper(ri.ins, mult_insts[2 * (k - DNB) + 1].ins,
                                               reason="WAR rec reuse")
                            # normalize: U_nat = op * (1/denom), per bank
                            for bank, (op, nhd) in enumerate(ops_):
                                in1 = (op[:, 0, 0:64 * nhd]
                                       .rearrange("p (s d) -> p s d", d=HD))
                                in2 = (rec[:, bank * 7:bank * 7 + nhd]
                                       .rearrange("p (s o) -> p s o", o=1))
                                b1, b2 = broadcast_tensor_aps(in1, in2)
                                mu = nc.vector.tensor_tensor(
                                    un[:, strip, bank * 7 * HD:(bank * 7 + nhd) * HD]
                                    .rearrange("p (s d) -> p s d", d=HD),
                                    b1, b2, OP.mult,
                                )
                                add_dep_helper(mu.ins, ri.ins, reason="RAW mult<-recip")
                                mult_insts.append(mu)

                        if b >= 6:
                            # last two batches: per-batch transposes so the
                            # projection tiles unlock one batch earlier
                            for kc in range(6):
                                utp = oppool.tile([128, 2, PC], bf16, tag="opx")
                                for ss in range(2):
                                    nc.tensor.transpose(
                                        utp[:, ss, :],
                                        un[:, ss, 128 * kc:128 * kc + 128],
                                        I128[0:PC, 0:PC],
                                    )
                                dst = (U_T[:, kc, b * N:(b + 1) * N]
                                       .rearrange("p (a b) -> p a b", a=2))
                                if kc % 2 == 1:
                                    nc.scalar.activation(dst, utp[:], AF.Identity)
                                else:
                                    nc.vector.tensor_copy(dst, utp[:])
                            emit_o_tiles((b + 1) * N)
                            prev_un = un
                            continue
                        # U_nat -> U_T transposes (pair-batched); copies on
                        # DVE except the very last pair (ACT idle by then)
                        if b % 2 == 1:
                            for kc in range(6):
                                utp = oppool.tile([128, 4, PC], bf16, tag="opx")
                                for j in range(4):
                                    bb = b - 1 + j // 2
                                    ss = j % 2
                                    src = un if bb == b else prev_un
                                    nc.tensor.transpose(
                                        utp[:, j, :],
                                        src[:, ss, 128 * kc:128 * kc + 128],
                                        I128[0:PC, 0:PC],
                                    )
                                dst = (U_T[:, kc, (b - 1) * N:(b + 1) * N]
                                       .rearrange("p (a b) -> p a b", a=4))
                                if b == 7 and kc % 2 == 1:
                                    nc.scalar.activation(dst, utp[:], AF.Identity)
                                else:
                                    nc.vector.tensor_copy(dst, utp[:])
                            o_ready[0] = max(o_ready[0], (b + 1) * N)
                            emit_o_one()
                        prev_un = un

                emit_o_tiles(TOK + 1)

            exptp_cm.__exit__(None, None, None)

            if DEBUG_DUMP:
                nc.sync.dma_start(dSg_d.ap(), Sg[:])
                # (dexpt dump removed: expt pool is closed by now)
                nc.sync.dma_start(dprm_d.ap(), prm[:])
                pass
                nc.sync.dma_start(dvpr_d.ap(), vpr[:, 0:2, :, :])
                nc.sync.dma_start(dUT_d.ap(), U_T[:, :, 0:N])
                nc.sync.dma_start(dqT_d.ap(), qT[:, :, 0:N])


    nc.compile()
    return nc


def _get_nc():
    if "nc" not in _NC_CACHE:
        _NC_CACHE["nc"] = _build_nc()
    return _NC_CACHE["nc"]


def _host_prep(inputs):
    x = np.asarray(inputs["x"], np.float32)
    W_qv = np.asarray(inputs["W_qv"], np.float32)
    k_ext = np.asarray(inputs["k_ext"], np.float32)
    attn_bias = np.asarray(inputs["attn_bias"], np.float32).reshape(1, N)
    gamma = np.asarray(inputs["bn_gamma"], np.float32).reshape(1, H)
    beta = np.asarray(inputs["bn_beta"], np.float32).reshape(1, H)
    W_proj = np.asarray(inputs["W_proj"], np.float32)
    b_proj = np.asarray(inputs["b_proj"], np.float32).reshape(1, C)

    F8 = ml_dtypes.float8_e4m3

    def dr_layout(a):  # [C, F] -> [128, 3, 2, F] DoubleRow k-tile layout
        return np.ascontiguousarray(
            a.reshape(3, 2, 128, -1).transpose(2, 0, 1, 3))

    # q path: weights scaled x64 (q lives in a 64x world; BN on scores is
    # scale-invariant given consistent stats, so nothing downstream changes
    # except the rsqrt newton seed)
    wq8 = dr_layout((64.0 * W_qv[:, :C]).astype(F8))
    # v path: 3-term fp8 residual split, 64x world, rescaled 1/64 at evac
    wv64 = 64.0 * W_qv[:, C:]
    wv8_q = wv64.astype(F8)
    wv8r_q = (wv64 - wv8_q.astype(np.float32)).astype(F8)
    wv8 = dr_layout(wv8_q)
    wv8r = dr_layout(wv8r_q)
    wp_bf = W_proj.astype(BF)
    kT1 = np.ascontiguousarray(k_ext.T).astype(BF)
    kT_bf = np.concatenate([kT1, kT1], axis=0)  # duplicated in both halves

    G = k_ext.astype(np.float64)
    G = G.T @ G
    L = np.linalg.cholesky(G + 1e-6 * np.eye(HD)).astype(np.float32)
    L2 = np.zeros((128, 128), np.float32)
    L2[0:64, 0:64] = L
    L2[64:128, 64:128] = L

    I128 = np.eye(128, dtype=np.float32)

    ksum = k_ext.sum(0).astype(np.float32)
    slhsA = np.zeros((128, 2), np.float32)
    slhsA[0:64, 0] = ksum
    slhsA[64:128, 1] = ksum

    sbias = np.ascontiguousarray(
        (SCALE * attn_bias.reshape(2, PC)).T
    ).astype(np.float32)

    R = W_proj.reshape(H, HD, C).sum(1).astype(np.float32)

    ndiv = np.concatenate([np.full(12, 1.0 / NA), np.full(12, 1.0 / NV)]
                          ).reshape(1, 24).astype(np.float32)
    gam2 = np.concatenate([gamma, gamma], axis=1).astype(np.float32)
    # attn scores live in the 64x world -> rstd seed is 64x smaller
    nseed = np.concatenate([np.full(12, 0.125 / 64.0), np.full(12, 1.0)]
                           ).reshape(1, 24).astype(np.float32)

    bpk = np.concatenate(
        [kT_bf, L2.astype(BF), I128.astype(BF)], axis=1)
    fpk = np.zeros((128, 4), np.float32)
    fpk[:, 0:2] = slhsA
    fpk[0:PC, 2:4] = sbias
    rpk = np.concatenate(
        [ndiv.reshape(-1), gam2.reshape(-1), nseed.reshape(-1),
         beta.reshape(-1), b_proj.reshape(-1)]).reshape(1, -1).astype(np.float32)
    common = dict(
        wq8=wq8, wv8=wv8, wv8r=wv8r, wp=wp_bf, bpk=bpk, fpk=fpk, rpk=rpk,
        R=R.astype(BF),
    )
    in_maps = []
    for c in range(8):
        xs = x[c * BL:(c + 1) * BL].reshape(TOK, C)
        xT = np.ascontiguousarray(xs.T).astype(np.float32)
        x8_q = xT.astype(F8)
        x8r_q = (xT - x8_q.astype(np.float32)).astype(F8)
        in_maps.append(dict(common, x8=dr_layout(x8_q), x8r=dr_layout(x8r_q)))
    return in_maps


def kernel(**inputs):
    from concourse.bass_utils import run_bass_kernel_spmd

    in_maps = _host_prep(inputs)
    nc = _get_nc()
    res = run_bass_kernel_spmd(nc, in_maps, core_ids=list(range(8)))
    outs = [res.results[c]["out"].reshape(BL, N, C) for c in range(8)]
    return np.concatenate(outs, axis=0)

